# revision 35
# baseline (speedup 1.0000x reference)
"""AllPassMORRCirculantConv2d on 8 TRN2 NeuronCores — data-parallel over batch.

Math (per spatial position, per output block):
    phase[p,q,k] = sum_j inten[q,j] * W[p,q,(k-j)%8]      (block circulant)
    T = mrr_tr(phase) = 1 - K/D,  D = c1 - c2*cos(phase)
    out[pk] = sum_q scale[q]*T[p,q,k]
Since sum_q scale[q] == 0 (differential rails), this reduces to
    out[pk] = sum_q (-K*scale[q]/c2) * 1/(a - cos(phase)),  a = c1/c2.

Per-core device pipeline (1 image = 4096 positions):
  x -> pad+square (ACT, gain folded into the square's pre-scale, bf16 out)
  -> 54 window-gather DMAs (im2col, win-major layout [96, 3*4096], bf16,
  split across the sync + gpsimd DMA queues) -> per 512-pos chunk: 18 bf16
  circulant matmuls (K=32 with zero-padded stationary halves so 16-row
  pairs stay 32-aligned) -> ACT Sin(pi/2 - phase) = cos -> one fused custom
  DVE op (affine + bit-trick seed + 1 Newton step = approx 1/(a-u), bf16
  out) -> 18 accumulating bf16 q-reduction matmuls (stationary = scaled
  identity blocks) -> ACT copy PSUM->SBUF -> DMA out.
"""

import contextlib
import ctypes
import sys
import types

import numpy as np

# ---- model constants -------------------------------------------------------
A = 0.8578
R = 0.8985
AR = A * R
C1 = 1.0 + AR * AR
C2 = 2.0 * AR
A_OVER = C1 / C2                      # pole position of 1/(a - cos)
KCONST = (1.0 - A * A) * (1.0 - R * R)
MORR_GAIN = (100.0 / 36.0) ** 0.5
SQG = float(MORR_GAIN ** 0.5)         # fold gain into the ACT square pre-scale

# Chebyshev-minimax seed pair for the bit-trick reciprocal (see dve_ops.py)
RECIP_C0 = -0.23549792
RECIP_C1 = 2.0017324

N_CORES = 8
NPAIR = 18                            # q-pairs (Q=36)
CHUNK = 512
NCHUNK = 8                            # 4096 positions / 512


def _install_ntff_hook():
    """Recreate antenv.axon_hooks + the ctypes NTFF profile hook the agent
    image lacks (mirrors trn_boot._ntff_profile_via_ctypes)."""
    if "antenv.axon_hooks" in sys.modules:
        return
    so_path = "/opt/axon/libaxon_pjrt.so"
    try:
        lib = ctypes.CDLL(so_path)
        lib.axon_start_nrt_profile.argtypes = [
            ctypes.POINTER(ctypes.c_int64),
            ctypes.c_size_t,
        ]
        lib.axon_start_nrt_profile.restype = ctypes.c_int64
        lib.axon_stop_nrt_profile.argtypes = [ctypes.c_char_p]
        lib.axon_stop_nrt_profile.restype = ctypes.c_int64
    except OSError:
        return

    @contextlib.contextmanager
    def _hook(output_dir, device_ids):
        import jax

        jax.devices()
        if device_ids:
            ids = (ctypes.c_int64 * len(device_ids))(*device_ids)
            rc = lib.axon_start_nrt_profile(ids, len(device_ids))
        else:
            rc = lib.axon_start_nrt_profile(None, 0)
        if rc != 0:
            raise RuntimeError(f"axon_start_nrt_profile rc={rc}")
        try:
            yield
        finally:
            n = lib.axon_stop_nrt_profile(str(output_dir).encode())
            print(f"ntff profile: {n} file(s) -> {output_dir}", file=sys.stderr)

    mod = types.ModuleType("antenv.axon_hooks")
    mod.set_axon_ntff_profile_hook = lambda h: None
    mod.get_axon_ntff_profile_hook = lambda: _hook
    sys.modules["antenv.axon_hooks"] = mod
    import antenv

    antenv.axon_hooks = mod


def _register_recip_op():
    """Register a fused custom DVE op:
        out = recip_seed_1NR(C2 - in0)   with C0/C1 the Chebyshev seed pair.
    6 ALU stages: rsub, bitwise_not, mul, mul, rsub, mul — fits the 8-slice
    pipe. ~0.4% max rel err; bf16 output makes the cast to the q-reduction
    matmul free."""
    from concourse import dve_ops
    from concourse.dve_spec import AluOp, Bin, C0, C1, C2, Spec, Src0, lower
    from concourse.dve_spec import _has_src1 as has_src1
    from concourse.dve_uop import DveOpSpec

    name = "RECIP_AFFINE_FAST_ANT"
    for op in dve_ops.OPS:
        if op.name == name:
            return op

    _D = C2 - Src0
    _not = Bin(AluOp.BITWISE_NOT, _D, _D)
    _y0 = _not * C0
    body = _y0 * (C1 - _D * _y0)

    def _ref(in0, in1, c0, c1, c2):
        D = (np.float32(c2) - in0.astype(np.float32)).astype(np.float32)
        nx = (~D.view(np.int32)).view(np.float32)
        y0 = nx * np.float32(c0)
        return (y0 * (np.float32(c1) - D * y0)).astype(np.float32)

    spec = Spec(body=body, reference=_ref)
    row = dve_ops._CUSTOM_DVE_ROW_BASE + len(dve_ops.OPS)
    assert row < 0x20
    shas = {}
    for ver in ("v3", "v4"):
        try:
            uops = lower(spec, ver=ver)
            shas[ver] = DveOpSpec(
                name=name, opcode=row, uops=uops, rd1_en=has_src1(spec)
            ).sha(ver)
        except Exception:
            pass
    op = dve_ops.DveOp(name, spec, subdim=False, uops_sha=shas)
    dve_ops.OPS.append(op)
    dve_ops.CUSTOM_DVE_SPECS[name] = spec
    dve_ops._SUB_OPCODE_FOR_NAME[name] = row
    return op


_NC_CACHE = {}


def build_nc():
    if "nc" in _NC_CACHE:
        return _NC_CACHE["nc"]
    import concourse.bacc as bacc
    import concourse.mybir as mybir
    import concourse.tile as tile

    f32 = mybir.dt.float32
    bf16 = mybir.dt.bfloat16
    AF = mybir.ActivationFunctionType

    recip_op = _register_recip_op()

    nc = bacc.Bacc("TRN2", target_bir_lowering=False)
    x_d = nc.dram_tensor("x", [32, 4096], f32, kind="ExternalInput")
    wblk_d = nc.dram_tensor("wblk", [96, 2304], bf16, kind="ExternalInput")
    sdual_d = nc.dram_tensor("sdual", [128, 1152], bf16, kind="ExternalInput")
    out_d = nc.dram_tensor("out", [64, 4096], f32, kind="ExternalOutput")

    PI_HALF = float(np.pi / 2)

    with tile.TileContext(nc) as tc:
        with (
            tc.tile_pool(name="const", bufs=1) as cpool,
            tc.tile_pool(name="u", bufs=3) as upool,
            tc.tile_pool(name="dv", bufs=3) as dpool,
            tc.tile_pool(name="ostage", bufs=2) as spool,
            tc.tile_pool(name="pps", bufs=2, space="PSUM") as ppool,
            tc.tile_pool(name="qps", bufs=2, space="PSUM") as qpool,
        ):
            wblk = cpool.tile([96, 2304], bf16)
            sdual = cpool.tile([128, 1152], bf16)
            pihalf = cpool.tile([128, 1], f32)
            scratch1 = cpool.tile([128, 1], f32)
            nc.vector.memset(pihalf[:], PI_HALF)
            xpad = cpool.tile([32, 4356], f32)
            xsq = cpool.tile([32, 4356], bf16)
            inten = cpool.tile([96, 12288], bf16)

            # force the trig table load off the critical path: the first
            # scalar op is a dep-free dummy Sin (square lives in the same
            # table set, so no reload later)
            nc.scalar.activation(scratch1[:], pihalf[:], AF.Sin,
                                 bias=pihalf[:], scale=-1.0)

            # zero only the pad border of the 66x66 frames
            xpad3 = xpad.rearrange("p (a b) -> p a b", b=66)
            nc.vector.memset(xpad3[:, 0, :], 0.0)        # top row
            nc.vector.memset(xpad3[:, 65, :], 0.0)       # bottom row
            nc.vector.memset(xpad3[:, 1:65, 0:1], 0.0)   # left col
            nc.vector.memset(xpad3[:, 1:65, 65:66], 0.0)  # right col
            x3 = x_d[:, :].rearrange("p (a b) -> p a b", b=64)
            xsq3w = xsq.rearrange("p (a b) -> p a b", b=66)
            # split the load + square into 4 row bands so the first window
            # gathers can start after ~1/4 of the image is squared; band 0
            # covers padded rows 0..17 = everything the first position
            # quarter's windows read
            BANDS = [(0, 18), (18, 34), (34, 50), (50, 66)]
            for r_lo, r_hi in BANDS:
                nc.sync.dma_start(
                    xpad3[:, max(1, r_lo):min(65, r_hi), 1:65],
                    x3[:, max(1, r_lo) - 1:min(65, r_hi) - 1, :],
                )
            nc.sync.dma_start(wblk[:], wblk_d[:, :])
            nc.sync.dma_start(sdual[:], sdual_d[:, :])
            for r_lo, r_hi in BANDS:
                # xsq = (sqrt(gain) * xpad)^2 = gain * x^2  (bf16 out)
                nc.scalar.activation(
                    xsq3w[:, r_lo:r_hi, :], xpad3[:, r_lo:r_hi, :],
                    AF.Square, bias=0.0, scale=SQG,
                )

            # im2col window gather: win = 9c + kk rows, positions on free dim.
            # h-major order (position quarters) so chunk 0's deps complete
            # first; alternate the sync/gpsimd queues.
            # slices of image rows: the first two are chunk-sized so chunk 0
            # unblocks as early as possible, the rest are quarter-sized
            HSLICES = [(0, 8), (8, 16), (16, 32), (32, 48), (48, 64)]
            xsq3 = xsq.rearrange("p (a b) -> p a b", b=66)
            inten4 = inten.rearrange("r (g a b) -> r g a b", g=3, b=64)
            n_dma = 0
            for hi, (row_lo, row_hi) in enumerate(HSLICES):
                for kk in range(9):
                    ki, kj = kk // 3, kk % 3
                    for g in range(3):
                        c_lo = max(0, -((-(96 * g - kk)) // 9))
                        c_hi = min(32, -((-(96 * g + 96 - kk)) // 9))
                        if c_hi <= c_lo:
                            continue
                        r0 = 9 * c_lo + kk - 96 * g
                        cnt = c_hi - c_lo
                        src = xsq3[c_lo:c_hi, ki + row_lo:ki + row_hi,
                                   kj:kj + 64]
                        dst = inten4[r0:r0 + 9 * (cnt - 1) + 1:9, g,
                                     row_lo:row_hi, :]
                        if hi < 2:
                            # prologue: scalar is idle -> use 3 DMA paths
                            eng = (nc.sync, nc.gpsimd,
                                   nc.scalar)[n_dma % 3]
                        else:
                            eng = nc.sync if n_dma % 2 == 0 else nc.gpsimd
                        eng.dma_start(dst, src)
                        n_dma += 1

            # main loop: 8 chunks of 512 positions
            for ch in range(NCHUNK):
                pos0 = CHUNK * ch
                qacc = qpool.tile([64, CHUNK], f32)
                n_qred = 0
                GW = 3 * CHUNK            # tri-group width
                for gb in range(3):
                    u2 = upool.tile([128, 2 * GW], f32)
                    d2 = dpool.tile([128, 2 * GW], bf16)
                    for par in range(2):
                        ts = [6 * gb + par + 2 * e for e in range(3)]
                        ph = ppool.tile([128, GW], f32)
                        for e, t in enumerate(ts):
                            nc.tensor.matmul(
                                ph[:, CHUNK * e:CHUNK * (e + 1)],
                                wblk[32 * e:32 * e + 32, 128 * t:128 * (t + 1)],
                                inten[32 * e:32 * e + 32,
                                      4096 * gb + pos0:4096 * gb + pos0 + CHUNK],
                                start=True, stop=True,
                            )
                        # u = sin(pi/2 - phase) = cos(phase)
                        nc.scalar.activation(u2[:, GW * par:GW * (par + 1)],
                                             ph[:], AF.Sin,
                                             bias=pihalf[:], scale=-1.0)
                    # d = approx 1/(A_OVER - u), bf16 out — one wide DVE op
                    nc.vector._custom_dve(
                        recip_op, out=d2[:], in0=u2[:],
                        s0=RECIP_C0, s1=RECIP_C1, imm2=A_OVER,
                    )
                    for par in range(2):
                        ts = [6 * gb + par + 2 * e for e in range(3)]
                        for e, t in enumerate(ts):
                            nc.tensor.matmul(
                                qacc[:, :],
                                sdual[:, 64 * t:64 * (t + 1)],
                                d2[:, GW * par + CHUNK * e:
                                    GW * par + CHUNK * (e + 1)],
                                start=(n_qred == 0),
                                stop=(n_qred == NPAIR - 1),
                                skip_group_check=True,
                            )
                            n_qred += 1
                out_sb = spool.tile([64, CHUNK], f32)
                nc.scalar.copy(out_sb[:], qacc[:])
                nc.sync.dma_start(out_d[:, pos0:pos0 + CHUNK], out_sb[:])

    nc.compile()
    _NC_CACHE["nc"] = nc
    return nc


def host_weights(weight, morr_output_scale):
    """Build the stationary operands on the host (tiny, deterministic)."""
    import ml_dtypes

    weight = np.asarray(weight, np.float32)
    mos = np.asarray(morr_output_scale, np.float32)

    # wblk[96, 2304]: circulant blocks. partition = (16t)%96 + 8*q2 + j,
    # col = 128t + 64*q2 + 8p + k, value = W[p, 2t+q2, (k-j)%8].
    wblk = np.zeros((96, 2304), np.float32)
    for t in range(NPAIR):
        base = (16 * t) % 96
        for q2 in range(2):
            q = 2 * t + q2
            for j in range(8):
                cols = (128 * t + 64 * q2 + 8 * np.arange(8)[:, None]
                        + np.arange(8)[None, :])
                wblk[base + 8 * q2 + j, cols.ravel()] = (
                    weight[:, q, (np.arange(8) - j) % 8].ravel())

    # sdual[128, 1152]: scaled identity blocks for the q-reduction.
    half = mos[:18]
    scale_ref = np.concatenate([half, -half])          # [36]
    scale_s = (-KCONST / C2) * scale_ref
    sdual = np.zeros((128, 1152), np.float32)
    for t in range(NPAIR):
        for q2 in range(2):
            rows = 64 * q2 + np.arange(64)
            sdual[rows, 64 * t + np.arange(64)] = scale_s[2 * t + q2]
    return wblk.astype(ml_dtypes.bfloat16), sdual.astype(ml_dtypes.bfloat16)


def make_in_maps(x, weight, morr_output_scale):
    x = np.asarray(x, np.float32)
    wblk, sdual = host_weights(weight, morr_output_scale)
    return [
        {
            "x": np.ascontiguousarray(x[b].reshape(32, 4096)),
            "wblk": wblk,
            "sdual": sdual,
        }
        for b in range(N_CORES)
    ]


def run(x, weight, morr_output_scale, trace=False, trace_kwargs=None):
    _install_ntff_hook()
    from concourse.bass_utils import run_bass_kernel_spmd

    nc = build_nc()
    in_maps = make_in_maps(x, weight, morr_output_scale)
    res = run_bass_kernel_spmd(
        nc, in_maps, core_ids=list(range(N_CORES)), trace=trace,
        **(trace_kwargs or {}),
    )
    out = np.stack(
        [res.results[b]["out"].reshape(64, 64, 64) for b in range(N_CORES)]
    ).astype(np.float32)
    return out, res


def kernel(x, weight, morr_output_scale):
    out, _ = run(x, weight, morr_output_scale, trace=False)
    return out


# revision 42
# speedup vs baseline: 1.2310x; 1.2310x over previous
"""AllPassMORRCirculantConv2d on 8 TRN2 NeuronCores — data-parallel over batch.

Math (per spatial position, per output block):
    phase[p,q,k] = sum_j inten[q,j] * W[p,q,(k-j)%8]      (block circulant)
    T = mrr_tr(phase) = 1 - K/D,  D = c1 - c2*cos(phase)
    out[pk] = sum_q scale[q]*T[p,q,k]
Since sum_q scale[q] == 0 (differential rails), this reduces to
    out[pk] = sum_q (-K*scale[q]/c2) * 1/(a - cos(phase)),  a = c1/c2.

Per-core device pipeline (1 image = 4096 positions):
  x -> pad+square in 4 row bands (ACT, gain folded into the square's
  pre-scale, bf16 out) -> 108 window-gather DMAs (im2col, win-major layout
  [96, 3*4096] bf16, 4 position-quarters x 27, spread over the
  sync/gpsimd/scalar DMA rings) -> per 512-pos chunk: 18 bf16 circulant
  matmuls (K=32 with zero-padded stationary halves so 16-row q-pairs stay
  32-aligned; 3 tri-grouped matmuls share a 3-bank PSUM tile) -> ACT
  Sin(pi/2 - phase) = cos -> one fused custom DVE op per 2 tri-groups
  (affine + bit-trick seed + 1 Newton step = approx 1/(a-u), bf16 out) ->
  18 accumulating bf16 q-reduction matmuls (stationary = scaled identity
  blocks) -> ACT copy PSUM->SBUF -> DMA out.

Measured on TRN2 (8 cores, data-parallel): ~124-131 us NEFF exec,
max-rel error ~6.3e-3 (gate 2e-2). Engine budget per core: ACT sin ~79us,
DVE recip ~80us (1x-rate floor), PE streams ~64us, all overlapped;
~7us framework preamble + ~25us im2col startup + ~10us drain tail.
"""

import contextlib
import ctypes
import sys
import types

import numpy as np

# ---- model constants -------------------------------------------------------
A = 0.8578
R = 0.8985
AR = A * R
C1 = 1.0 + AR * AR
C2 = 2.0 * AR
A_OVER = C1 / C2                      # pole position of 1/(a - cos)
KCONST = (1.0 - A * A) * (1.0 - R * R)
MORR_GAIN = (100.0 / 36.0) ** 0.5
SQG = float(MORR_GAIN ** 0.5)         # fold gain into the ACT square pre-scale

# Chebyshev-minimax seed pair for the bit-trick reciprocal (see dve_ops.py)
RECIP_C0 = -0.23549792
RECIP_C1 = 2.0017324

N_CORES = 8
NPAIR = 18                            # q-pairs (Q=36)
CHUNK = 512
NCHUNK = 8                            # 4096 positions / 512


def _install_ntff_hook():
    """Recreate antenv.axon_hooks + the ctypes NTFF profile hook the agent
    image lacks (mirrors trn_boot._ntff_profile_via_ctypes)."""
    if "antenv.axon_hooks" in sys.modules:
        return
    so_path = "/opt/axon/libaxon_pjrt.so"
    try:
        lib = ctypes.CDLL(so_path)
        lib.axon_start_nrt_profile.argtypes = [
            ctypes.POINTER(ctypes.c_int64),
            ctypes.c_size_t,
        ]
        lib.axon_start_nrt_profile.restype = ctypes.c_int64
        lib.axon_stop_nrt_profile.argtypes = [ctypes.c_char_p]
        lib.axon_stop_nrt_profile.restype = ctypes.c_int64
    except OSError:
        return

    @contextlib.contextmanager
    def _hook(output_dir, device_ids):
        import jax

        jax.devices()
        if device_ids:
            ids = (ctypes.c_int64 * len(device_ids))(*device_ids)
            rc = lib.axon_start_nrt_profile(ids, len(device_ids))
        else:
            rc = lib.axon_start_nrt_profile(None, 0)
        if rc != 0:
            raise RuntimeError(f"axon_start_nrt_profile rc={rc}")
        try:
            yield
        finally:
            n = lib.axon_stop_nrt_profile(str(output_dir).encode())
            print(f"ntff profile: {n} file(s) -> {output_dir}", file=sys.stderr)

    mod = types.ModuleType("antenv.axon_hooks")
    mod.set_axon_ntff_profile_hook = lambda h: None
    mod.get_axon_ntff_profile_hook = lambda: _hook
    sys.modules["antenv.axon_hooks"] = mod
    import antenv

    antenv.axon_hooks = mod


def _register_recip_op():
    """Register a fused custom DVE op:
        out = recip_seed_1NR(C2 - in0)   with C0/C1 the Chebyshev seed pair.
    6 ALU stages: rsub, bitwise_not, mul, mul, rsub, mul — fits the 8-slice
    pipe. ~0.4% max rel err; bf16 output makes the cast to the q-reduction
    matmul free."""
    from concourse import dve_ops
    from concourse.dve_spec import AluOp, Bin, C0, C1, C2, Spec, Src0, lower
    from concourse.dve_spec import _has_src1 as has_src1
    from concourse.dve_uop import DveOpSpec

    name = "RECIP_AFFINE_FAST_ANT"
    for op in dve_ops.OPS:
        if op.name == name:
            return op

    _D = C2 - Src0
    _not = Bin(AluOp.BITWISE_NOT, _D, _D)
    _y0 = _not * C0
    body = _y0 * (C1 - _D * _y0)

    def _ref(in0, in1, c0, c1, c2):
        D = (np.float32(c2) - in0.astype(np.float32)).astype(np.float32)
        nx = (~D.view(np.int32)).view(np.float32)
        y0 = nx * np.float32(c0)
        return (y0 * (np.float32(c1) - D * y0)).astype(np.float32)

    spec = Spec(body=body, reference=_ref)
    row = dve_ops._CUSTOM_DVE_ROW_BASE + len(dve_ops.OPS)
    assert row < 0x20
    shas = {}
    for ver in ("v3", "v4"):
        try:
            uops = lower(spec, ver=ver)
            shas[ver] = DveOpSpec(
                name=name, opcode=row, uops=uops, rd1_en=has_src1(spec)
            ).sha(ver)
        except Exception:
            pass
    op = dve_ops.DveOp(name, spec, subdim=False, uops_sha=shas)
    dve_ops.OPS.append(op)
    dve_ops.CUSTOM_DVE_SPECS[name] = spec
    dve_ops._SUB_OPCODE_FOR_NAME[name] = row
    return op


_NC_CACHE = {}


def build_nc():
    if "nc" in _NC_CACHE:
        return _NC_CACHE["nc"]
    import concourse.bacc as bacc
    import concourse.mybir as mybir
    import concourse.tile as tile

    f32 = mybir.dt.float32
    bf16 = mybir.dt.bfloat16
    AF = mybir.ActivationFunctionType

    recip_op = _register_recip_op()

    nc = bacc.Bacc("TRN2", target_bir_lowering=False)
    x_d = nc.dram_tensor("x", [32, 4096], f32, kind="ExternalInput")
    wblk_d = nc.dram_tensor("wblk", [96, 2304], bf16, kind="ExternalInput")
    sdual_d = nc.dram_tensor("sdual", [128, 1152], bf16, kind="ExternalInput")
    out_d = nc.dram_tensor("out", [64, 4096], f32, kind="ExternalOutput")

    PI_HALF = float(np.pi / 2)

    with tile.TileContext(nc) as tc:
        with (
            tc.tile_pool(name="const", bufs=1) as cpool,
            tc.tile_pool(name="u", bufs=3) as upool,
            tc.tile_pool(name="dv", bufs=3) as dpool,
            tc.tile_pool(name="ostage", bufs=2) as spool,
            tc.tile_pool(name="pps", bufs=2, space="PSUM") as ppool,
            tc.tile_pool(name="qps", bufs=2, space="PSUM") as qpool,
        ):
            wblk = cpool.tile([96, 2304], bf16)
            sdual = cpool.tile([128, 1152], bf16)
            pihalf = cpool.tile([128, 1], f32)
            scratch1 = cpool.tile([128, 1], f32)
            nc.vector.memset(pihalf[:], PI_HALF)
            xpad = cpool.tile([32, 4356], f32)
            xsq = cpool.tile([32, 4356], bf16)
            inten = cpool.tile([96, 12288], bf16)

            # force the trig table load off the critical path: the first
            # scalar op is a dep-free dummy Sin (square lives in the same
            # table set, so no reload later)
            nc.scalar.activation(scratch1[:], pihalf[:], AF.Sin,
                                 bias=pihalf[:], scale=-1.0)

            # zero only the pad border of the 66x66 frames
            xpad3 = xpad.rearrange("p (a b) -> p a b", b=66)
            nc.vector.memset(xpad3[:, 0, :], 0.0)        # top row
            nc.vector.memset(xpad3[:, 65, :], 0.0)       # bottom row
            nc.vector.memset(xpad3[:, 1:65, 0:1], 0.0)   # left col
            nc.vector.memset(xpad3[:, 1:65, 65:66], 0.0)  # right col
            x3 = x_d[:, :].rearrange("p (a b) -> p a b", b=64)
            xsq3w = xsq.rearrange("p (a b) -> p a b", b=66)
            # split the load + square into 4 row bands so the first window
            # gathers can start after ~1/4 of the image is squared; band 0
            # covers padded rows 0..17 = everything the first position
            # quarter's windows read
            BANDS = [(0, 18), (18, 34), (34, 50), (50, 66)]
            for r_lo, r_hi in BANDS:
                nc.sync.dma_start(
                    xpad3[:, max(1, r_lo):min(65, r_hi), 1:65],
                    x3[:, max(1, r_lo) - 1:min(65, r_hi) - 1, :],
                )
            nc.gpsimd.dma_start(wblk[:], wblk_d[:, :])
            nc.gpsimd.dma_start(sdual[:], sdual_d[:, :])
            for r_lo, r_hi in BANDS:
                # xsq = (sqrt(gain) * xpad)^2 = gain * x^2  (bf16 out)
                nc.scalar.activation(
                    xsq3w[:, r_lo:r_hi, :], xpad3[:, r_lo:r_hi, :],
                    AF.Square, bias=0.0, scale=SQG,
                )

            # im2col window gather: win = 9c + kk rows, positions on free dim.
            # h-major order (position quarters) so chunk 0's deps complete
            # first; alternate the sync/gpsimd queues.
            # slices of image rows: the first two are chunk-sized so chunk 0
            # unblocks as early as possible, the rest are quarter-sized
            HSLICES = [(0, 16), (16, 32), (32, 48), (48, 64)]
            xsq3 = xsq.rearrange("p (a b) -> p a b", b=66)
            inten4 = inten.rearrange("r (g a b) -> r g a b", g=3, b=64)
            n_dma = 0
            for hi, (row_lo, row_hi) in enumerate(HSLICES):
                for kk in range(9):
                    ki, kj = kk // 3, kk % 3
                    for g in range(3):
                        c_lo = max(0, -((-(96 * g - kk)) // 9))
                        c_hi = min(32, -((-(96 * g + 96 - kk)) // 9))
                        if c_hi <= c_lo:
                            continue
                        r0 = 9 * c_lo + kk - 96 * g
                        cnt = c_hi - c_lo
                        src = xsq3[c_lo:c_hi, ki + row_lo:ki + row_hi,
                                   kj:kj + 64]
                        dst = inten4[r0:r0 + 9 * (cnt - 1) + 1:9, g,
                                     row_lo:row_hi, :]
                        if hi < 1:
                            # prologue: scalar is idle -> use 3 DMA paths,
                            # spread each g-block across all rings so the
                            # first tri-group's deps finish earliest
                            eng = (nc.sync, nc.gpsimd,
                                   nc.scalar)[(kk + g) % 3]
                        else:
                            eng = nc.sync if n_dma % 2 == 0 else nc.gpsimd
                        eng.dma_start(dst, src)
                        n_dma += 1

            # main loop: 8 chunks of 512 positions
            SEGS = [(CHUNK * ch, CHUNK) for ch in range(NCHUNK)]
            for pos0, cw in SEGS:
                qacc = qpool.tile([64, CHUNK], f32, tag="qacc")
                n_qred = 0
                gw = 3 * cw               # tri-group width
                for gb in range(3):
                    u2 = upool.tile([128, 6 * CHUNK], f32, tag="u2")
                    d2 = dpool.tile([128, 6 * CHUNK], bf16, tag="d2")
                    for par in range(2):
                        ts = [6 * gb + par + 2 * e for e in range(3)]
                        ph = ppool.tile([128, 3 * CHUNK], f32, tag="ph")
                        for e, t in enumerate(ts):
                            # each matmul writes the start of its own PSUM
                            # bank (outputs must be bank-aligned)
                            nc.tensor.matmul(
                                ph[:, CHUNK * e:CHUNK * e + cw],
                                wblk[32 * e:32 * e + 32, 128 * t:128 * (t + 1)],
                                inten[32 * e:32 * e + 32,
                                      4096 * gb + pos0:4096 * gb + pos0 + cw],
                                start=True, stop=True,
                            )
                        # u = sin(pi/2 - phase) = cos(phase)
                        ph3 = ph.rearrange("p (e c) -> p e c", e=3)
                        nc.scalar.activation(u2[:, gw * par:gw * (par + 1)],
                                             ph3[:, :, 0:cw], AF.Sin,
                                             bias=pihalf[:], scale=-1.0)
                    # d = approx 1/(A_OVER - u), bf16 out — one wide DVE op
                    nc.vector._custom_dve(
                        recip_op, out=d2[:, 0:2 * gw], in0=u2[:, 0:2 * gw],
                        s0=RECIP_C0, s1=RECIP_C1, imm2=A_OVER,
                    )
                    for par in range(2):
                        ts = [6 * gb + par + 2 * e for e in range(3)]
                        for e, t in enumerate(ts):
                            nc.tensor.matmul(
                                qacc[:, 0:cw],
                                sdual[:, 64 * t:64 * (t + 1)],
                                d2[:, gw * par + cw * e:gw * par + cw * (e + 1)],
                                start=(n_qred == 0),
                                stop=(n_qred == NPAIR - 1),
                                skip_group_check=True,
                            )
                            n_qred += 1
                out_sb = spool.tile([64, CHUNK], f32, tag="osb")
                nc.scalar.copy(out_sb[:, 0:cw], qacc[:, 0:cw])
                nc.sync.dma_start(out_d[:, pos0:pos0 + cw], out_sb[:, 0:cw])

    nc.compile()
    _NC_CACHE["nc"] = nc
    return nc


def host_weights(weight, morr_output_scale):
    """Build the stationary operands on the host (tiny, deterministic)."""
    import ml_dtypes

    weight = np.asarray(weight, np.float32)
    mos = np.asarray(morr_output_scale, np.float32)

    # wblk[96, 2304]: circulant blocks. partition = (16t)%96 + 8*q2 + j,
    # col = 128t + 64*q2 + 8p + k, value = W[p, 2t+q2, (k-j)%8].
    wblk = np.zeros((96, 2304), np.float32)
    for t in range(NPAIR):
        base = (16 * t) % 96
        for q2 in range(2):
            q = 2 * t + q2
            for j in range(8):
                cols = (128 * t + 64 * q2 + 8 * np.arange(8)[:, None]
                        + np.arange(8)[None, :])
                wblk[base + 8 * q2 + j, cols.ravel()] = (
                    weight[:, q, (np.arange(8) - j) % 8].ravel())

    # sdual[128, 1152]: scaled identity blocks for the q-reduction.
    half = mos[:18]
    scale_ref = np.concatenate([half, -half])          # [36]
    scale_s = (-KCONST / C2) * scale_ref
    sdual = np.zeros((128, 1152), np.float32)
    for t in range(NPAIR):
        for q2 in range(2):
            rows = 64 * q2 + np.arange(64)
            sdual[rows, 64 * t + np.arange(64)] = scale_s[2 * t + q2]
    return wblk.astype(ml_dtypes.bfloat16), sdual.astype(ml_dtypes.bfloat16)


def make_in_maps(x, weight, morr_output_scale):
    x = np.asarray(x, np.float32)
    wblk, sdual = host_weights(weight, morr_output_scale)
    return [
        {
            "x": np.ascontiguousarray(x[b].reshape(32, 4096)),
            "wblk": wblk,
            "sdual": sdual,
        }
        for b in range(N_CORES)
    ]


def run(x, weight, morr_output_scale, trace=False, trace_kwargs=None):
    _install_ntff_hook()
    from concourse.bass_utils import run_bass_kernel_spmd

    nc = build_nc()
    in_maps = make_in_maps(x, weight, morr_output_scale)
    res = run_bass_kernel_spmd(
        nc, in_maps, core_ids=list(range(N_CORES)), trace=trace,
        **(trace_kwargs or {}),
    )
    out = np.stack(
        [res.results[b]["out"].reshape(64, 64, 64) for b in range(N_CORES)]
    ).astype(np.float32)
    return out, res


def kernel(x, weight, morr_output_scale):
    out, _ = run(x, weight, morr_output_scale, trace=False)
    return out


# revision 51
# speedup vs baseline: 1.2386x; 1.0062x over previous
"""AllPassMORRCirculantConv2d on 8 TRN2 NeuronCores — data-parallel over batch.

Math (per spatial position, per output block):
    phase[p,q,k] = sum_j inten[q,j] * W[p,q,(k-j)%8]      (block circulant)
    T = mrr_tr(phase) = 1 - K/D,  D = c1 - c2*cos(phase)
    out[pk] = sum_q scale[q]*T[p,q,k]
Since sum_q scale[q] == 0 (differential rails), this reduces to
    out[pk] = sum_q (-K*scale[q]/c2) * 1/(a - cos(phase)),  a = c1/c2.

Per-core device pipeline (1 image = 4096 positions):
  x -> pad+square in 4 row bands (ACT, gain folded into the square's
  pre-scale, bf16 out) -> 108 window-gather DMAs (im2col, win-major layout
  [96, 3*4096] bf16, 4 position-quarters x 27, spread over the
  sync/gpsimd/scalar DMA rings) -> per 512-pos chunk: 18 bf16 circulant
  matmuls (K=32 with zero-padded stationary halves so 16-row q-pairs stay
  32-aligned; 3 tri-grouped matmuls share a 3-bank PSUM tile) -> ACT
  Sin(pi/2 - phase) = cos -> one fused custom DVE op per 2 tri-groups
  (affine + bit-trick seed + 1 Newton step = approx 1/(a-u), bf16 out) ->
  18 accumulating bf16 q-reduction matmuls (stationary = scaled identity
  blocks) -> ACT copy PSUM->SBUF -> DMA out.

Measured on TRN2 (8 cores, data-parallel): ~124-131 us NEFF exec,
max-rel error ~6.3e-3 (gate 2e-2). Engine budget per core: ACT sin ~79us,
DVE recip ~80us (1x-rate floor), PE streams ~64us, all overlapped;
~7us framework preamble + ~25us im2col startup + ~10us drain tail.
"""

import contextlib
import ctypes
import sys
import types

import numpy as np

# ---- model constants -------------------------------------------------------
A = 0.8578
R = 0.8985
AR = A * R
C1 = 1.0 + AR * AR
C2 = 2.0 * AR
A_OVER = C1 / C2                      # pole position of 1/(a - cos)
KCONST = (1.0 - A * A) * (1.0 - R * R)
MORR_GAIN = (100.0 / 36.0) ** 0.5
SQG = float(MORR_GAIN ** 0.5)         # fold gain into the ACT square pre-scale

# Chebyshev-minimax seed pair for the bit-trick reciprocal (see dve_ops.py)
RECIP_C0 = -0.23549792
RECIP_C1 = 2.0017324

N_CORES = 8
NPAIR = 18                            # q-pairs (Q=36)
CHUNK = 512
NCHUNK = 8                            # 4096 positions / 512


def _install_ntff_hook():
    """Recreate antenv.axon_hooks + the ctypes NTFF profile hook the agent
    image lacks (mirrors trn_boot._ntff_profile_via_ctypes)."""
    if "antenv.axon_hooks" in sys.modules:
        return
    so_path = "/opt/axon/libaxon_pjrt.so"
    try:
        lib = ctypes.CDLL(so_path)
        lib.axon_start_nrt_profile.argtypes = [
            ctypes.POINTER(ctypes.c_int64),
            ctypes.c_size_t,
        ]
        lib.axon_start_nrt_profile.restype = ctypes.c_int64
        lib.axon_stop_nrt_profile.argtypes = [ctypes.c_char_p]
        lib.axon_stop_nrt_profile.restype = ctypes.c_int64
    except OSError:
        return

    @contextlib.contextmanager
    def _hook(output_dir, device_ids):
        import jax

        jax.devices()
        if device_ids:
            ids = (ctypes.c_int64 * len(device_ids))(*device_ids)
            rc = lib.axon_start_nrt_profile(ids, len(device_ids))
        else:
            rc = lib.axon_start_nrt_profile(None, 0)
        if rc != 0:
            raise RuntimeError(f"axon_start_nrt_profile rc={rc}")
        try:
            yield
        finally:
            n = lib.axon_stop_nrt_profile(str(output_dir).encode())
            print(f"ntff profile: {n} file(s) -> {output_dir}", file=sys.stderr)

    mod = types.ModuleType("antenv.axon_hooks")
    mod.set_axon_ntff_profile_hook = lambda h: None
    mod.get_axon_ntff_profile_hook = lambda: _hook
    sys.modules["antenv.axon_hooks"] = mod
    import antenv

    antenv.axon_hooks = mod


def _register_recip_op():
    """Register a fused custom DVE op:
        out = recip_seed_1NR(C2 - in0)   with C0/C1 the Chebyshev seed pair.
    6 ALU stages: rsub, bitwise_not, mul, mul, rsub, mul — fits the 8-slice
    pipe. ~0.4% max rel err; bf16 output makes the cast to the q-reduction
    matmul free."""
    from concourse import dve_ops
    from concourse.dve_spec import AluOp, Bin, C0, C1, C2, Spec, Src0, lower
    from concourse.dve_spec import _has_src1 as has_src1
    from concourse.dve_uop import DveOpSpec

    name = "RECIP_AFFINE_FAST_ANT"
    for op in dve_ops.OPS:
        if op.name == name:
            return op

    _D = C2 - Src0
    _not = Bin(AluOp.BITWISE_NOT, _D, _D)
    _y0 = _not * C0
    body = _y0 * (C1 - _D * _y0)

    def _ref(in0, in1, c0, c1, c2):
        D = (np.float32(c2) - in0.astype(np.float32)).astype(np.float32)
        nx = (~D.view(np.int32)).view(np.float32)
        y0 = nx * np.float32(c0)
        return (y0 * (np.float32(c1) - D * y0)).astype(np.float32)

    spec = Spec(body=body, reference=_ref)
    row = dve_ops._CUSTOM_DVE_ROW_BASE + len(dve_ops.OPS)
    assert row < 0x20
    shas = {}
    for ver in ("v3", "v4"):
        try:
            uops = lower(spec, ver=ver)
            shas[ver] = DveOpSpec(
                name=name, opcode=row, uops=uops, rd1_en=has_src1(spec)
            ).sha(ver)
        except Exception:
            pass
    op = dve_ops.DveOp(name, spec, subdim=False, uops_sha=shas)
    dve_ops.OPS.append(op)
    dve_ops.CUSTOM_DVE_SPECS[name] = spec
    dve_ops._SUB_OPCODE_FOR_NAME[name] = row
    return op


_NC_CACHE = {}


def build_nc(qred_behind=False):
    key = ("nc", qred_behind)
    if key in _NC_CACHE:
        return _NC_CACHE[key]
    import concourse.bacc as bacc
    import concourse.mybir as mybir
    import concourse.tile as tile

    f32 = mybir.dt.float32
    bf16 = mybir.dt.bfloat16
    AF = mybir.ActivationFunctionType

    recip_op = _register_recip_op()

    nc = bacc.Bacc("TRN2", target_bir_lowering=False)
    x_d = nc.dram_tensor("x", [32, 4096], f32, kind="ExternalInput")
    wblk_d = nc.dram_tensor("wblk", [96, 2304], bf16, kind="ExternalInput")
    sdual_d = nc.dram_tensor("sdual", [128, 1152], bf16, kind="ExternalInput")
    out_d = nc.dram_tensor("out", [64, 4096], f32, kind="ExternalOutput")

    PI_HALF = float(np.pi / 2)

    with tile.TileContext(nc) as tc:
        with (
            tc.tile_pool(name="const", bufs=1) as cpool,
            tc.tile_pool(name="u", bufs=3) as upool,
            tc.tile_pool(name="dv", bufs=3) as dpool,
            tc.tile_pool(name="ostage", bufs=2) as spool,
            tc.tile_pool(name="pps", bufs=2, space="PSUM") as ppool,
            tc.tile_pool(name="qps", bufs=2, space="PSUM") as qpool,
        ):
            wblk = cpool.tile([96, 2304], bf16)
            sdual = cpool.tile([128, 1152], bf16)
            pihalf = cpool.tile([128, 1], f32)
            scratch1 = cpool.tile([128, 1], f32)
            nc.vector.memset(pihalf[:], PI_HALF)
            xpad = cpool.tile([32, 4356], f32)
            xsq = cpool.tile([32, 4356], bf16)
            inten = cpool.tile([96, 12288], bf16)

            # force the trig table load off the critical path: the first
            # scalar op is a dep-free dummy Sin (square lives in the same
            # table set, so no reload later)
            nc.scalar.activation(scratch1[:], pihalf[:], AF.Sin,
                                 bias=pihalf[:], scale=-1.0)

            # zero only the pad border of the 66x66 frames
            xpad3 = xpad.rearrange("p (a b) -> p a b", b=66)
            nc.vector.memset(xpad3[:, 0, :], 0.0)        # top row
            nc.vector.memset(xpad3[:, 65, :], 0.0)       # bottom row
            nc.vector.memset(xpad3[:, 1:65, 0:1], 0.0)   # left col
            nc.vector.memset(xpad3[:, 1:65, 65:66], 0.0)  # right col
            x3 = x_d[:, :].rearrange("p (a b) -> p a b", b=64)
            xsq3w = xsq.rearrange("p (a b) -> p a b", b=66)
            # split the load + square into 4 row bands so the first window
            # gathers can start after ~1/4 of the image is squared; band 0
            # covers padded rows 0..17 = everything the first position
            # quarter's windows read
            BANDS = [(0, 18), (18, 34), (34, 50), (50, 66)]
            for r_lo, r_hi in BANDS:
                nc.sync.dma_start(
                    xpad3[:, max(1, r_lo):min(65, r_hi), 1:65],
                    x3[:, max(1, r_lo) - 1:min(65, r_hi) - 1, :],
                )
            nc.gpsimd.dma_start(wblk[:], wblk_d[:, :])
            nc.gpsimd.dma_start(sdual[:], sdual_d[:, :])
            for r_lo, r_hi in BANDS:
                # xsq = (sqrt(gain) * xpad)^2 = gain * x^2  (bf16 out)
                nc.scalar.activation(
                    xsq3w[:, r_lo:r_hi, :], xpad3[:, r_lo:r_hi, :],
                    AF.Square, bias=0.0, scale=SQG,
                )

            # im2col window gather: win = 9c + kk rows, positions on free dim.
            # h-major order (position quarters) so chunk 0's deps complete
            # first; alternate the sync/gpsimd queues.
            # position-quarter slices of image rows
            HSLICES = [(0, 16), (16, 32), (32, 48), (48, 64)]
            xsq3 = xsq.rearrange("p (a b) -> p a b", b=66)
            inten4 = inten.rearrange("r (g a b) -> r g a b", g=3, b=64)
            n_dma = 0
            for hi, (row_lo, row_hi) in enumerate(HSLICES):
                for kk in range(9):
                    ki, kj = kk // 3, kk % 3
                    for g in range(3):
                        c_lo = max(0, -((-(96 * g - kk)) // 9))
                        c_hi = min(32, -((-(96 * g + 96 - kk)) // 9))
                        if c_hi <= c_lo:
                            continue
                        r0 = 9 * c_lo + kk - 96 * g
                        cnt = c_hi - c_lo
                        src = xsq3[c_lo:c_hi, ki + row_lo:ki + row_hi,
                                   kj:kj + 64]
                        dst = inten4[r0:r0 + 9 * (cnt - 1) + 1:9, g,
                                     row_lo:row_hi, :]
                        if hi < 1:
                            # prologue: scalar is idle -> use 3 DMA paths,
                            # spread each g-block across all rings so the
                            # first tri-group's deps finish earliest
                            eng = (nc.sync, nc.gpsimd,
                                   nc.scalar)[(kk + g) % 3]
                        else:
                            eng = nc.sync if n_dma % 2 == 0 else nc.gpsimd
                        eng.dma_start(dst, src)
                        n_dma += 1

            # main loop: 8 chunks of 512 positions
            SEGS = [(CHUNK * ch, CHUNK) for ch in range(NCHUNK)]
            for pos0, cw in SEGS:
                qacc = qpool.tile([64, CHUNK], f32, tag="qacc")
                n_qred = 0
                gw = 3 * cw               # tri-group width
                pending = []              # (d2, gb) whose q-reduction is due

                def emit_qred(d2, gb):
                    nonlocal n_qred
                    for par in range(2):
                        ts = [6 * gb + par + 2 * e for e in range(3)]
                        for e, t in enumerate(ts):
                            nc.tensor.matmul(
                                qacc[:, 0:cw],
                                sdual[:, 64 * t:64 * (t + 1)],
                                d2[:, gw * par + cw * e:gw * par + cw * (e + 1)],
                                start=(n_qred == 0),
                                stop=(n_qred == NPAIR - 1),
                                skip_group_check=True,
                            )
                            n_qred += 1

                for gb in range(3):
                    u2 = upool.tile([128, 6 * CHUNK], f32, tag="u2")
                    d2 = dpool.tile([128, 6 * CHUNK], bf16, tag="d2")
                    for par in range(2):
                        ts = [6 * gb + par + 2 * e for e in range(3)]
                        ph = ppool.tile([128, 3 * CHUNK], f32, tag="ph")
                        for e, t in enumerate(ts):
                            # each matmul writes the start of its own PSUM
                            # bank (outputs must be bank-aligned)
                            nc.tensor.matmul(
                                ph[:, CHUNK * e:CHUNK * e + cw],
                                wblk[32 * e:32 * e + 32, 128 * t:128 * (t + 1)],
                                inten[32 * e:32 * e + 32,
                                      4096 * gb + pos0:4096 * gb + pos0 + cw],
                                start=True, stop=True,
                            )
                        # u = sin(pi/2 - phase) = cos(phase)
                        ph3 = ph.rearrange("p (e c) -> p e c", e=3)
                        nc.scalar.activation(u2[:, gw * par:gw * (par + 1)],
                                             ph3[:, :, 0:cw], AF.Sin,
                                             bias=pihalf[:], scale=-1.0)
                    # d = approx 1/(A_OVER - u), bf16 out — one wide DVE op
                    nc.vector._custom_dve(
                        recip_op, out=d2[:, 0:2 * gw], in0=u2[:, 0:2 * gw],
                        s0=RECIP_C0, s1=RECIP_C1, imm2=A_OVER,
                    )
                    # emit the q-reduction one group behind so the PE stream
                    # has the next group's phase matmuls ahead of the
                    # recip-gated q-reduction
                    pending.append((d2, gb))
                    if len(pending) > (1 if qred_behind else 0):
                        emit_qred(*pending.pop(0))
                for d2p, gbp in pending:
                    emit_qred(d2p, gbp)
                out_sb = spool.tile([64, CHUNK], f32, tag="osb")
                nc.scalar.copy(out_sb[:, 0:cw], qacc[:, 0:cw])
                nc.sync.dma_start(out_d[:, pos0:pos0 + cw], out_sb[:, 0:cw])

    nc.compile()
    _NC_CACHE[key] = nc
    return nc


def host_weights(weight, morr_output_scale):
    """Build the stationary operands on the host (tiny, deterministic)."""
    import ml_dtypes

    weight = np.asarray(weight, np.float32)
    mos = np.asarray(morr_output_scale, np.float32)

    # wblk[96, 2304]: circulant blocks. partition = (16t)%96 + 8*q2 + j,
    # col = 128t + 64*q2 + 8p + k, value = W[p, 2t+q2, (k-j)%8].
    wblk = np.zeros((96, 2304), np.float32)
    for t in range(NPAIR):
        base = (16 * t) % 96
        for q2 in range(2):
            q = 2 * t + q2
            for j in range(8):
                cols = (128 * t + 64 * q2 + 8 * np.arange(8)[:, None]
                        + np.arange(8)[None, :])
                wblk[base + 8 * q2 + j, cols.ravel()] = (
                    weight[:, q, (np.arange(8) - j) % 8].ravel())

    # sdual[128, 1152]: scaled identity blocks for the q-reduction.
    half = mos[:18]
    scale_ref = np.concatenate([half, -half])          # [36]
    scale_s = (-KCONST / C2) * scale_ref
    sdual = np.zeros((128, 1152), np.float32)
    for t in range(NPAIR):
        for q2 in range(2):
            rows = 64 * q2 + np.arange(64)
            sdual[rows, 64 * t + np.arange(64)] = scale_s[2 * t + q2]
    return wblk.astype(ml_dtypes.bfloat16), sdual.astype(ml_dtypes.bfloat16)


def make_in_maps(x, weight, morr_output_scale):
    x = np.asarray(x, np.float32)
    wblk, sdual = host_weights(weight, morr_output_scale)
    return [
        {
            "x": np.ascontiguousarray(x[b].reshape(32, 4096)),
            "wblk": wblk,
            "sdual": sdual,
        }
        for b in range(N_CORES)
    ]


def run(x, weight, morr_output_scale, trace=False, trace_kwargs=None,
        qred_behind=False):
    _install_ntff_hook()
    from concourse.bass_utils import run_bass_kernel_spmd

    nc = build_nc(qred_behind)
    in_maps = make_in_maps(x, weight, morr_output_scale)
    res = run_bass_kernel_spmd(
        nc, in_maps, core_ids=list(range(N_CORES)), trace=trace,
        **(trace_kwargs or {}),
    )
    out = np.stack(
        [res.results[b]["out"].reshape(64, 64, 64) for b in range(N_CORES)]
    ).astype(np.float32)
    return out, res


def kernel(x, weight, morr_output_scale):
    out, _ = run(x, weight, morr_output_scale, trace=False)
    return out
